# Initial kernel scaffold
#
"""Multi-head attention (B=2, T=2048, D=1024, H=16, causal) on 8 TRN2 cores.

Sharding: 2-way data parallel on batch x 4-way tensor parallel on heads.
Core c handles batch c//4, heads [4*(c%4), 4*(c%4)+4). Each core computes
its Q/K/V projections, causal attention + softmax for its 4 heads, and a
partial output projection over its 256 d_model columns of W_o; the host
sums the 4 partials per batch (the "all-reduce").

Outputs per core: its (4, T, T) slice of attention weights (upper triangle
left unwritten - output buffers are zero-initialized by the runtime) and a
(T, D) partial output.
"""

from contextlib import ExitStack

import numpy as np

B, T, D, H = 2, 2048, 1024, 16
DK = D // H  # 64
NCORES = 8
HG = 4  # head-group shards
HPG = H // HG  # heads per core = 4
FS = HPG * DK  # feature slice per core = 256
NQT = T // 128  # 16 query tiles
NTT = T // 512  # 4 token 512-chunks
ND = D // 128  # 8 contraction chunks
SCALE = 0.125  # 1/sqrt(DK)
NEG = -1.0e9

_CACHE = {}


def _build():
    import concourse.bass as bass  # noqa: F401
    import concourse.mybir as mybir
    import concourse.tile as tile
    from concourse import bacc
    from concourse.masks import make_identity

    F32 = mybir.dt.float32
    AF = mybir.ActivationFunctionType
    AX = mybir.AxisListType

    nc = bacc.Bacc("TRN2", target_bir_lowering=False, debug=False,
                   num_devices=NCORES)

    xqT = nc.dram_tensor("xqT", (D, T), F32, kind="ExternalInput").ap()
    xkT = nc.dram_tensor("xkT", (D, T), F32, kind="ExternalInput").ap()
    xvT = nc.dram_tensor("xvT", (D, T), F32, kind="ExternalInput").ap()
    wT = nc.dram_tensor("wT", (D, 3 * FS), F32, kind="ExternalInput").ap()
    woT = nc.dram_tensor("woT", (FS, D), F32, kind="ExternalInput").ap()
    diag = nc.dram_tensor("diag", (128, 128), F32, kind="ExternalInput").ap()
    attn = nc.dram_tensor("attn", (HPG, T, T), F32, kind="ExternalOutput").ap()
    outp = nc.dram_tensor("outp", (T, D), F32, kind="ExternalOutput").ap()

    with tile.TileContext(nc) as tc, ExitStack() as ctx:
        # ---- persistent SBUF ----
        persist = ctx.enter_context(tc.tile_pool(name="persist", bufs=1))
        # feature-major Q^T,K^T: 2 tiles of [128, T] each (features 0-127 / 128-255)
        qT_sb = [persist.tile([128, T], F32, tag=f"qT{i}") for i in range(2)]
        kT_sb = [persist.tile([128, T], F32, tag=f"kT{i}") for i in range(2)]
        # token-major V: tile per 128-token block, [128, FS]
        v_sb = persist.tile([128, NQT * FS], F32, tag="v")
        # normalized attn output (token-major), per 128-token block [128, FS]
        comb = persist.tile([128, NQT * FS], F32, tag="comb")
        wo_sb = persist.tile([128, 2 * D], F32, tag="wo")  # [FS=2*128, D]
        ident = persist.tile([128, 128], F32, tag="ident")
        diag_sb = persist.tile([128, 128], F32, tag="diag")

        make_identity(nc, ident[:])
        nc.sync.dma_start(diag_sb[:], diag)
        for i in range(2):
            nc.sync.dma_start(wo_sb[:, i * D:(i + 1) * D],
                              woT[i * 128:(i + 1) * 128, :])

        # ---- phase 1: projections ----
        with ExitStack() as p1:
            wpool = p1.enter_context(tc.tile_pool(name="wqkv", bufs=1))
            xpool = p1.enter_context(tc.tile_pool(name="xstream", bufs=2))
            pps = p1.enter_context(
                tc.tile_pool(name="projps", bufs=3, space="PSUM"))

            w_sb = wpool.tile([128, ND * 3 * FS], F32)  # [d 8x128, 768]
            for d in range(ND):
                nc.sync.dma_start(
                    w_sb[:, d * 3 * FS:(d + 1) * 3 * FS],
                    wT[d * 128:(d + 1) * 128, :])

            for tt in range(NTT):  # 512-token chunks
                ts512 = slice(tt * 512, (tt + 1) * 512)
                xq_t = xpool.tile([128, ND * 512], F32, tag="xq")
                xk_t = xpool.tile([128, ND * 512], F32, tag="xk")
                xv_t = xpool.tile([128, ND * 512], F32, tag="xv")
                for d in range(ND):
                    dsl = slice(d * 512, (d + 1) * 512)
                    nc.sync.dma_start(xq_t[:, dsl], xqT[d * 128:(d + 1) * 128, ts512])
                    nc.sync.dma_start(xk_t[:, dsl], xkT[d * 128:(d + 1) * 128, ts512])
                    nc.sync.dma_start(xv_t[:, dsl], xvT[d * 128:(d + 1) * 128, ts512])
                # Q^T,K^T feature-major: [128f, 512t] blocks
                for dst, x_t, woff in ((qT_sb, xq_t, 0), (kT_sb, xk_t, FS)):
                    for ft in range(2):
                        ps = pps.tile([128, 512], F32, tag="qkps")
                        for d in range(ND):
                            nc.tensor.matmul(
                                ps[:],
                                w_sb[:, d * 3 * FS + woff + ft * 128:
                                     d * 3 * FS + woff + (ft + 1) * 128],
                                x_t[:, d * 512:(d + 1) * 512],
                                start=(d == 0), stop=(d == ND - 1))
                        nc.vector.tensor_copy(dst[ft][:, ts512], ps[:])
                # V token-major: [128t, 256f] blocks
                for s in range(4):
                    tb = tt * 4 + s  # 128-token block index
                    ps = pps.tile([128, FS], F32, tag="vps")
                    for d in range(ND):
                        nc.tensor.matmul(
                            ps[:],
                            xv_t[:, d * 512 + s * 128:d * 512 + (s + 1) * 128],
                            w_sb[:, d * 3 * FS + 2 * FS:d * 3 * FS + 3 * FS],
                            start=(d == 0), stop=(d == ND - 1))
                    nc.vector.tensor_copy(v_sb[:, tb * FS:(tb + 1) * FS], ps[:])

        # ---- phase 2+3: attention + output projection ----
        with ExitStack() as p2:
            sps = p2.enter_context(tc.tile_pool(name="sps", bufs=2, space="PSUM"))
            tps = p2.enter_context(tc.tile_pool(name="tps", bufs=2, space="PSUM"))
            aps = p2.enter_context(tc.tile_pool(name="aps", bufs=2, space="PSUM"))
            ops = p2.enter_context(tc.tile_pool(name="ops", bufs=2, space="PSUM"))
            rows = p2.enter_context(tc.tile_pool(name="rows", bufs=2))
            small = p2.enter_context(tc.tile_pool(name="small", bufs=3))
            ostage = p2.enter_context(tc.tile_pool(name="ostage", bufs=2))

            for qt in range(NQT):
                q0 = qt * 128
                width = (qt + 1) * 128  # causal row width
                nchunk = (width + 511) // 512
                for h in range(HPG):
                    fq = h * DK  # feature offset of head h in the 256 slice
                    lq = qT_sb[fq // 128][fq % 128:fq % 128 + DK, q0:q0 + 128]
                    a_row = rows.tile([128, T], F32, tag="a_row")
                    w_row = rows.tile([128, T], F32, tag="w_row")
                    sums = small.tile([128, 4], F32, tag="sums")
                    attn_ps = aps.tile([128, DK], F32, tag="attn")

                    # scores + exp per 512-chunk
                    for c in range(nchunk):
                        k0 = c * 512
                        cw = min(512, width - k0)
                        ps = sps.tile([128, 512], F32, tag="score")
                        nc.tensor.matmul(
                            ps[:, :cw],
                            lq,
                            kT_sb[fq // 128][fq % 128:fq % 128 + DK, k0:k0 + cw],
                            start=True, stop=True)
                        if c == nchunk - 1:
                            # causal mask on the diagonal 128-block
                            nc.vector.tensor_add(
                                ps[:, cw - 128:cw], ps[:, cw - 128:cw], diag_sb[:])
                        nc.scalar.activation(
                            a_row[:, k0:k0 + cw], ps[:, :cw], AF.Exp,
                            scale=SCALE, accum_out=sums[:, c:c + 1])

                    stot = small.tile([128, 1], F32, tag="stot")
                    if nchunk > 1:
                        nc.vector.reduce_sum(stot[:], sums[:, :nchunk], axis=AX.X)
                    else:
                        nc.vector.tensor_copy(stot[:], sums[:, 0:1])
                    recip = small.tile([128, 1], F32, tag="recip")
                    nc.vector.reciprocal(recip[:], stot[:])

                    # normalized weights -> DRAM (upper triangle stays 0)
                    nc.vector.tensor_scalar_mul(
                        w_row[:, :width], a_row[:, :width], recip[:])
                    nc.sync.dma_start(
                        attn[h, q0:q0 + 128, 0:width], w_row[:, :width])

                    # A^T via PE transpose (batched per 512), then A@V
                    for c in range(nchunk):
                        k0 = c * 512
                        cw = min(512, width - k0)
                        nt = cw // 128
                        tp = tps.tile([128, 512], F32, tag="trans")
                        for i in range(nt):
                            nc.tensor.transpose(
                                tp[:, i * 128:(i + 1) * 128],
                                a_row[:, k0 + i * 128:k0 + (i + 1) * 128],
                                ident[:])
                        at_sb = small.tile([128, 512], F32, tag="at")
                        nc.vector.tensor_copy(at_sb[:, :cw], tp[:, :cw])
                        for i in range(nt):
                            kb = (k0 + i * 128) // 128
                            nc.tensor.matmul(
                                attn_ps[:],
                                at_sb[:, i * 128:(i + 1) * 128],
                                v_sb[:, kb * FS + fq:kb * FS + fq + DK],
                                start=(c == 0 and i == 0),
                                stop=(c == nchunk - 1 and i == nt - 1))

                    # normalize + store into comb
                    nc.vector.tensor_scalar_mul(
                        comb[:, qt * FS + fq:qt * FS + fq + DK],
                        attn_ps[:], recip[:])

                # phase 3: partial output projection for this token block
                ctp = tps.tile([128, 256], F32, tag="trans")
                for dc in range(2):
                    nc.tensor.transpose(
                        ctp[:, dc * 128:(dc + 1) * 128],
                        comb[:, qt * FS + dc * 128:qt * FS + (dc + 1) * 128],
                        ident[:])
                ct_sb = small.tile([128, 256], F32, tag="ct")
                nc.vector.tensor_copy(ct_sb[:], ctp[:])
                out_sb = ostage.tile([128, D], F32, tag="out")
                for n in range(2):
                    ps = ops.tile([128, 512], F32, tag="ops")
                    for dc in range(2):
                        nc.tensor.matmul(
                            ps[:],
                            ct_sb[:, dc * 128:(dc + 1) * 128],
                            wo_sb[:, dc * D + n * 512:dc * D + (n + 1) * 512],
                            start=(dc == 0), stop=(dc == 1))
                    nc.vector.tensor_copy(out_sb[:, n * 512:(n + 1) * 512], ps[:])
                nc.sync.dma_start(outp[q0:q0 + 128, :], out_sb[:])

    nc.compile()
    return nc


def kernel(query, key, value, mask, Wq, Wk, Wv, Wo):
    import concourse.bass_utils as bass_utils

    if "nc" not in _CACHE:
        _CACHE["nc"] = _build()
    nc = _CACHE["nc"]

    query = np.ascontiguousarray(query, dtype=np.float32)
    key = np.ascontiguousarray(key, dtype=np.float32)
    value = np.ascontiguousarray(value, dtype=np.float32)

    diag = np.where(np.arange(128)[None, :] > np.arange(128)[:, None],
                    np.float32(NEG), np.float32(0.0)).astype(np.float32)

    in_maps = []
    for c in range(NCORES):
        b, g = divmod(c, HG)
        fs = slice(g * FS, (g + 1) * FS)
        wT = np.concatenate(
            [Wq[fs].T, Wk[fs].T, Wv[fs].T], axis=1)  # (D, 3*FS)
        in_maps.append({
            "xqT": np.ascontiguousarray(query[b].T),
            "xkT": np.ascontiguousarray(key[b].T),
            "xvT": np.ascontiguousarray(value[b].T),
            "wT": np.ascontiguousarray(wT),
            "woT": np.ascontiguousarray(Wo[:, fs].T),
            "diag": diag,
        })

    res = bass_utils.run_bass_kernel_spmd(
        nc, in_maps, core_ids=list(range(NCORES)))

    attn_w = np.empty((B, H, T, T), dtype=np.float32)
    output = np.zeros((B, T, D), dtype=np.float32)
    for c in range(NCORES):
        b, g = divmod(c, HG)
        attn_w[b, g * HPG:(g + 1) * HPG] = res.results[c]["attn"]
        output[b] += res.results[c]["outp"]
    return output, attn_w


# revision 4
# speedup vs baseline: 8.5442x; 8.5442x over previous
"""Multi-head attention (B=2, T=2048, D=1024, H=16, causal) on 8 TRN2 cores.

Sharding: 2-way data parallel on batch x 4-way tensor parallel on heads.
Core c handles batch c//4, heads [4*(c%4), 4*(c%4)+4). Each core computes
its Q/K/V projections, causal attention + softmax for its 4 heads, and a
partial output projection over its 256 d_model columns of W_o; the host
sums the 4 partials per batch (the "all-reduce").

Outputs per core: its (4, T, T) slice of attention weights (upper triangle
left unwritten - output buffers are zero-initialized by the runtime) and a
(T, D) partial output.
"""

from contextlib import ExitStack

import numpy as np

B, T, D, H = 2, 2048, 1024, 16
DK = D // H  # 64
NCORES = 8
HG = 4  # head-group shards
HPG = H // HG  # heads per core = 4
FS = HPG * DK  # feature slice per core = 256
NQT = T // 128  # 16 query tiles
NTT = T // 512  # 4 token 512-chunks
ND = D // 128  # 8 contraction chunks
SCALE = 0.125  # 1/sqrt(DK)
NEG = -1.0e9

_CACHE = {}


def _build():
    import concourse.bass as bass  # noqa: F401
    import concourse.mybir as mybir
    import concourse.tile as tile
    from concourse import bacc
    from concourse.masks import make_identity

    F32 = mybir.dt.float32
    AF = mybir.ActivationFunctionType
    AX = mybir.AxisListType

    nc = bacc.Bacc("TRN2", target_bir_lowering=False, debug=False,
                   num_devices=NCORES)

    xqT = nc.dram_tensor("xqT", (D, T), F32, kind="ExternalInput").ap()
    xkT = nc.dram_tensor("xkT", (D, T), F32, kind="ExternalInput").ap()
    xvT = nc.dram_tensor("xvT", (D, T), F32, kind="ExternalInput").ap()
    wT = nc.dram_tensor("wT", (D, 3 * FS), F32, kind="ExternalInput").ap()
    woT = nc.dram_tensor("woT", (FS, D), F32, kind="ExternalInput").ap()
    diag = nc.dram_tensor("diag", (128, 128), F32, kind="ExternalInput").ap()
    attn = nc.dram_tensor("attn", (HPG, T, T), F32, kind="ExternalOutput").ap()
    outp = nc.dram_tensor("outp", (T, D), F32, kind="ExternalOutput").ap()

    with tile.TileContext(nc) as tc, ExitStack() as ctx:
        # ---- persistent SBUF ----
        persist = ctx.enter_context(tc.tile_pool(name="persist", bufs=1))
        # feature-major Q^T,K^T: 2 tiles of [128, T] each (features 0-127 / 128-255)
        qT_sb = [persist.tile([128, T], F32, tag=f"qT{i}", name=f"qT{i}") for i in range(2)]
        kT_sb = [persist.tile([128, T], F32, tag=f"kT{i}", name=f"kT{i}") for i in range(2)]
        # token-major V: tile per 128-token block, [128, FS]
        v_sb = persist.tile([128, NQT * FS], F32, tag="v")
        # normalized attn output (token-major), per 128-token block [128, FS]
        comb = persist.tile([128, NQT * FS], F32, tag="comb")
        wo_sb = persist.tile([128, 2 * D], F32, tag="wo")  # [FS=2*128, D]
        ident = persist.tile([128, 128], F32, tag="ident")
        diag_sb = persist.tile([128, 128], F32, tag="diag")

        make_identity(nc, ident[:])
        nc.sync.dma_start(diag_sb[:], diag)
        for i in range(2):
            nc.sync.dma_start(wo_sb[:, i * D:(i + 1) * D],
                              woT[i * 128:(i + 1) * 128, :])

        # ---- phase 1: projections ----
        with ExitStack() as p1:
            wpool = p1.enter_context(tc.tile_pool(name="wqkv", bufs=1))
            xpool = p1.enter_context(tc.tile_pool(name="xstream", bufs=2))
            pps = p1.enter_context(
                tc.tile_pool(name="projps", bufs=3, space="PSUM"))

            w_sb = wpool.tile([128, ND * 3 * FS], F32)  # [d 8x128, 768]
            for d in range(ND):
                nc.sync.dma_start(
                    w_sb[:, d * 3 * FS:(d + 1) * 3 * FS],
                    wT[d * 128:(d + 1) * 128, :])

            for tt in range(NTT):  # 512-token chunks
                ts512 = slice(tt * 512, (tt + 1) * 512)
                xq_t = xpool.tile([128, ND * 512], F32, tag="xq")
                xk_t = xpool.tile([128, ND * 512], F32, tag="xk")
                xv_t = xpool.tile([128, ND * 512], F32, tag="xv")
                for d in range(ND):
                    dsl = slice(d * 512, (d + 1) * 512)
                    nc.sync.dma_start(xq_t[:, dsl], xqT[d * 128:(d + 1) * 128, ts512])
                    nc.sync.dma_start(xk_t[:, dsl], xkT[d * 128:(d + 1) * 128, ts512])
                    nc.sync.dma_start(xv_t[:, dsl], xvT[d * 128:(d + 1) * 128, ts512])
                # Q^T,K^T feature-major: [128f, 512t] blocks
                for dst, x_t, woff in ((qT_sb, xq_t, 0), (kT_sb, xk_t, FS)):
                    for ft in range(2):
                        ps = pps.tile([128, 512], F32, tag="qkps")
                        for d in range(ND):
                            nc.tensor.matmul(
                                ps[:],
                                w_sb[:, d * 3 * FS + woff + ft * 128:
                                     d * 3 * FS + woff + (ft + 1) * 128],
                                x_t[:, d * 512:(d + 1) * 512],
                                start=(d == 0), stop=(d == ND - 1))
                        nc.vector.tensor_copy(dst[ft][:, ts512], ps[:])
                # V token-major: [128t, 256f] blocks
                for s in range(4):
                    tb = tt * 4 + s  # 128-token block index
                    ps = pps.tile([128, FS], F32, tag="vps")
                    for d in range(ND):
                        nc.tensor.matmul(
                            ps[:],
                            xv_t[:, d * 512 + s * 128:d * 512 + (s + 1) * 128],
                            w_sb[:, d * 3 * FS + 2 * FS:d * 3 * FS + 3 * FS],
                            start=(d == 0), stop=(d == ND - 1))
                    nc.vector.tensor_copy(v_sb[:, tb * FS:(tb + 1) * FS], ps[:])

        # ---- phase 2+3: attention + output projection ----
        with ExitStack() as p2:
            sps = p2.enter_context(tc.tile_pool(name="sps", bufs=2, space="PSUM"))
            tps = p2.enter_context(tc.tile_pool(name="tps", bufs=2, space="PSUM"))
            aps = p2.enter_context(tc.tile_pool(name="aps", bufs=2, space="PSUM"))
            ops = p2.enter_context(tc.tile_pool(name="ops", bufs=2, space="PSUM"))
            rows = p2.enter_context(tc.tile_pool(name="rows", bufs=2))
            small = p2.enter_context(tc.tile_pool(name="small", bufs=3))
            ostage = p2.enter_context(tc.tile_pool(name="ostage", bufs=2))

            for qt in range(NQT):
                q0 = qt * 128
                width = (qt + 1) * 128  # causal row width
                nchunk = (width + 511) // 512
                for h in range(HPG):
                    fq = h * DK  # feature offset of head h in the 256 slice
                    lq = qT_sb[fq // 128][fq % 128:fq % 128 + DK, q0:q0 + 128]
                    a_row = rows.tile([128, T], F32, tag="a_row")
                    w_row = rows.tile([128, T], F32, tag="w_row")
                    sums = small.tile([128, 4], F32, tag="sums")
                    attn_ps = aps.tile([128, DK], F32, tag="attn")

                    # scores + exp per 512-chunk
                    for c in range(nchunk):
                        k0 = c * 512
                        cw = min(512, width - k0)
                        ps = sps.tile([128, 512], F32, tag="score")
                        nc.tensor.matmul(
                            ps[:, :cw],
                            lq,
                            kT_sb[fq // 128][fq % 128:fq % 128 + DK, k0:k0 + cw],
                            start=True, stop=True)
                        if c == nchunk - 1:
                            # causal mask on the diagonal 128-block
                            nc.vector.tensor_add(
                                ps[:, cw - 128:cw], ps[:, cw - 128:cw], diag_sb[:])
                        nc.scalar.activation(
                            a_row[:, k0:k0 + cw], ps[:, :cw], AF.Exp,
                            scale=SCALE, accum_out=sums[:, c:c + 1])

                    stot = small.tile([128, 1], F32, tag="stot")
                    if nchunk > 1:
                        nc.vector.reduce_sum(stot[:], sums[:, :nchunk], axis=AX.X)
                    else:
                        nc.vector.tensor_copy(stot[:], sums[:, 0:1])
                    recip = small.tile([128, 1], F32, tag="recip")
                    nc.vector.reciprocal(recip[:], stot[:])

                    # normalized weights -> DRAM (upper triangle stays 0)
                    nc.vector.tensor_scalar_mul(
                        w_row[:, :width], a_row[:, :width], recip[:])
                    nc.sync.dma_start(
                        attn[h, q0:q0 + 128, 0:width], w_row[:, :width])

                    # A^T via PE transpose (batched per 512), then A@V
                    for c in range(nchunk):
                        k0 = c * 512
                        cw = min(512, width - k0)
                        nt = cw // 128
                        tp = tps.tile([128, 512], F32, tag="trans")
                        for i in range(nt):
                            nc.tensor.transpose(
                                tp[:, i * 128:(i + 1) * 128],
                                a_row[:, k0 + i * 128:k0 + (i + 1) * 128],
                                ident[:])
                        at_sb = small.tile([128, 512], F32, tag="at")
                        nc.vector.tensor_copy(at_sb[:, :cw], tp[:, :cw])
                        for i in range(nt):
                            kb = (k0 + i * 128) // 128
                            nc.tensor.matmul(
                                attn_ps[:],
                                at_sb[:, i * 128:(i + 1) * 128],
                                v_sb[:, kb * FS + fq:kb * FS + fq + DK],
                                start=(c == 0 and i == 0),
                                stop=(c == nchunk - 1 and i == nt - 1))

                    # normalize + store into comb
                    nc.vector.tensor_scalar_mul(
                        comb[:, qt * FS + fq:qt * FS + fq + DK],
                        attn_ps[:], recip[:])

                # phase 3: partial output projection for this token block
                ctp = tps.tile([128, 256], F32, tag="trans")
                for dc in range(2):
                    nc.tensor.transpose(
                        ctp[:, dc * 128:(dc + 1) * 128],
                        comb[:, qt * FS + dc * 128:qt * FS + (dc + 1) * 128],
                        ident[:])
                ct_sb = small.tile([128, 256], F32, tag="ct")
                nc.vector.tensor_copy(ct_sb[:], ctp[:])
                out_sb = ostage.tile([128, D], F32, tag="out")
                for n in range(2):
                    ps = ops.tile([128, 512], F32, tag="ops")
                    for dc in range(2):
                        nc.tensor.matmul(
                            ps[:],
                            ct_sb[:, dc * 128:(dc + 1) * 128],
                            wo_sb[:, dc * D + n * 512:dc * D + (n + 1) * 512],
                            start=(dc == 0), stop=(dc == 1))
                    nc.vector.tensor_copy(out_sb[:, n * 512:(n + 1) * 512], ps[:])
                nc.sync.dma_start(outp[q0:q0 + 128, :], out_sb[:])

    nc.compile()
    return nc


def _prep_in_maps(query, key, value, Wq, Wk, Wv, Wo):
    query = np.ascontiguousarray(query, dtype=np.float32)
    key = np.ascontiguousarray(key, dtype=np.float32)
    value = np.ascontiguousarray(value, dtype=np.float32)

    diag = np.where(np.arange(128)[None, :] > np.arange(128)[:, None],
                    np.float32(NEG), np.float32(0.0)).astype(np.float32)

    xT = {}
    for b in range(B):
        xT[b] = (np.ascontiguousarray(query[b].T),
                 np.ascontiguousarray(key[b].T),
                 np.ascontiguousarray(value[b].T))
    in_maps = []
    for c in range(NCORES):
        b, g = divmod(c, HG)
        fs = slice(g * FS, (g + 1) * FS)
        wT = np.concatenate(
            [Wq[fs].T, Wk[fs].T, Wv[fs].T], axis=1)  # (D, 3*FS)
        in_maps.append({
            "xqT": xT[b][0],
            "xkT": xT[b][1],
            "xvT": xT[b][2],
            "wT": np.ascontiguousarray(wT),
            "woT": np.ascontiguousarray(Wo[:, fs].T),
            "diag": diag,
        })
    return in_maps


def kernel(query, key, value, mask, Wq, Wk, Wv, Wo):
    import concourse.bass_utils as bass_utils

    if "nc" not in _CACHE:
        _CACHE["nc"] = _build()
    nc = _CACHE["nc"]

    in_maps = _prep_in_maps(query, key, value, Wq, Wk, Wv, Wo)

    import time as _time
    _t0 = _time.time()
    res = bass_utils.run_bass_kernel_spmd(
        nc, in_maps, core_ids=list(range(NCORES)))
    _CACHE["last_res"] = res
    _CACHE["spmd_time"] = _time.time() - _t0

    attn_w = np.empty((B, H, T, T), dtype=np.float32)
    output = np.zeros((B, T, D), dtype=np.float32)
    for c in range(NCORES):
        b, g = divmod(c, HG)
        attn_w[b, g * HPG:(g + 1) * HPG] = res.results[c]["attn"]
        output[b] += res.results[c]["outp"]
    return output, attn_w
